# revision 1
# baseline (speedup 1.0000x reference)
"""Trainium2 Bass kernel for nn_Attention_89172110999574.

Strategy (8 NeuronCores, data parallel — 1 batch element per core):
  - x transposed on-chip via PE; QKV projections as matmuls.
  - Scores computed TRANSPOSED (ST[j,i] = k_j . q_i) so that softmax
    reduction rides the attn@V matmul: rhs is exp(ST), lhsT is [v | 1],
    giving the softmax denominator as an extra output row for free.
  - Relative-position bias handled with zero gathers: the bias matrix is
    block-Toeplitz (block (ri,rj) depends only on |ri-rj|), so a per-head
    strip table MS[(g,cj), u, ci] = E_h[|u-31-g|, |ci-cj|] is built once
    with a handful of strided DMAs; every score tile's bias is then a
    contiguous strided slice of MS added into PSUM via one identity
    matmul (scaled identity folds the 1/scale**2 factor).
  - exp() without max-subtraction (scores are ~N(0,1); |s|<~8 so exp is
    safe in fp32), gelu deferred to a single phase to avoid ACT
    table-set switches.
  - fp32r (full-rate fp32 PE mode) for all big matmuls.
"""

import os
import sys

import numpy as np

for _p in ("/opt/trn_rl_repo", "/root/.axon_site/_ro/trn_rl_repo"):
    if os.path.isdir(_p) and _p not in sys.path:
        sys.path.insert(0, _p)

import concourse.bass as bass
import concourse.tile as tile
from concourse import mybir
from concourse.bass_utils import run_bass_kernel_spmd
from concourse.masks import make_identity

N = 1024          # tokens per batch (32*32)
D = 256           # model dim
H = 8             # heads
DK = 32           # head dim (qk)
DV = 64           # head dim (v)
DOUT = 256        # output dim
NCORES = 8
FM = 32           # fmap
SCALE = float(DK) ** -0.5          # 1/sqrt(32)
BN_C = float(1.0 / np.sqrt(1.0 + 1e-5))
F32 = mybir.dt.float32
F32R = mybir.dt.float32r

USE_F32R = True
# matmul-operand dtype: float32r tiles (producers round); F32 fallback
MDT = F32R if USE_F32R else F32


def _r(ap):
    """matmul operands are already MDT-typed tiles."""
    return ap


def build_nc():
    nc = bass.Bass("TRN2", target_bir_lowering=False, debug=False)

    x = nc.dram_tensor("x", [N, D], F32, kind="ExternalInput").ap()
    wq = nc.dram_tensor("wq", [D, H * DK], F32, kind="ExternalInput").ap()
    wk = nc.dram_tensor("wk", [D, H * DK], F32, kind="ExternalInput").ap()
    wv = nc.dram_tensor("wv", [D, H * DV], F32, kind="ExternalInput").ap()
    wo = nc.dram_tensor("wo", [H * DV, DOUT], F32, kind="ExternalInput").ap()
    pe = nc.dram_tensor("pe", [N, H], F32, kind="ExternalInput").ap()
    bo = nc.dram_tensor("bo", [DOUT], F32, kind="ExternalInput").ap()
    gam = nc.dram_tensor("gam", [DOUT], F32, kind="ExternalInput").ap()
    bet = nc.dram_tensor("bet", [DOUT], F32, kind="ExternalInput").ap()
    out = nc.dram_tensor("out", [N, DOUT], F32, kind="ExternalOutput").ap()

    # scratch DRAM: per-head |s-31| expansion of pos_emb rows
    wfullh = nc.dram_tensor("wfullh", [H, 32, 63], F32R).ap()
    # scratch DRAM for partition-broadcasting the softmax recip rows
    rrd = nc.dram_tensor("rrd", [H, N], F32).ap()

    with tile.TileContext(nc) as tc:
        with (
            tc.tile_pool(name="const", bufs=1) as constp,
            tc.tile_pool(name="big", bufs=1) as bigp,
            tc.tile_pool(name="xin", bufs=3) as xinp,
            tc.tile_pool(name="exps", bufs=3) as expp,
            tc.tile_pool(name="small", bufs=2) as smallp,
            tc.tile_pool(name="yout", bufs=3) as youtp,
            tc.tile_pool(name="ps1", bufs=2, space="PSUM") as ps1p,
            tc.tile_pool(name="ps2", bufs=2, space="PSUM") as ps2p,
        ):
            # ---------------- constants / tables -----------------------
            ident = constp.tile([128, 128], F32)
            make_identity(nc, ident)
            isc = constp.tile([128, 128], MDT)
            # identity scaled by 1/scale^2 = 32: folds bias/scale into PSUM add
            nc.scalar.mul(isc, ident, float(DK))

            # Build wfull[h, t, s] = pos_emb[32*|t-31| + |s-31|, h].
            # 1) pos_emb -> SBUF E_sb[a, b, h] (contiguous)
            e_sb = smallp.tile([32, 32, 8], F32, tag="e_sb")
            nc.sync.dma_start(
                out=e_sb,
                in_=bass.AP(tensor=pe.tensor, offset=0,
                            ap=[[32 * H, 32], [H, 32], [1, 32 * H // 32]]),
            )
            # 2) s-flip on DVE: wrow[a, h, s] = E[a, |s-31|, h]
            wrow = smallp.tile([32, 8, 63], F32R, tag="wrow")
            nc.vector.tensor_copy(
                wrow[:, :, 0:31],
                bass.AP(tensor=e_sb.tensor, offset=e_sb.offset + 31 * 8,
                        ap=[e_sb.ap[0], [1, 8], [-8, 31]]),
            )
            nc.vector.tensor_copy(
                wrow[:, :, 31:63],
                bass.AP(tensor=e_sb.tensor, offset=e_sb.offset,
                        ap=[e_sb.ap[0], [1, 8], [8, 32]]),
            )
            # 3) dump wrow to DRAM: wfullh[h, a, s] = E_h[a, |s-31|]
            for h in range(H):
                nc.sync.dma_start(out=wfullh[h], in_=wrow[:, h, :])

            # 4) UWsb[cj, h, a, ci] = E_h[a, |ci-cj|] = wfullh[h, a, 31-cj+ci]
            #    one DMA per cj (all-positive strides, contiguous last dim)
            ms = bigp.tile([128, H, 66, 32], F32R)
            with tc.tile_pool(name="uw", bufs=1) as uwp:
                uwsb = uwp.tile([32, H, 32, 32], F32R)
                # alternate DMA queues to halve the gather wall time
                for cj in range(32):
                    eng = nc.sync if cj % 2 == 0 else nc.gpsimd
                    eng.dma_start(
                        out=uwsb[cj:cj + 1],
                        in_=bass.AP(tensor=wfullh.tensor, offset=31 - cj,
                                    ap=[[2016, 8], [63, 32], [1, 32]]),
                    )
                # 5) u-expansion into MS[(g,cj), h, u, ci] = E_h[|u-31-g|,|ci-cj|]
                #    upper half u=31+g..62+g plain; lower half u=g..30+g reads
                #    uwsb with a descending (negative mid-dim step is legal)
                # head-major order: head h's bias slices become ready after
                # its own 8 fills, letting phase C start ~8x earlier; gpsimd
                # queue keeps this off the sync-DMA queue
                for h in range(H):
                    for g in range(4):
                        nc.sync.dma_start(
                            out=ms[32 * g:32 * (g + 1), h, 31 + g:63 + g, :],
                            in_=uwsb[:, h, :, :],
                        )
                        nc.gpsimd.dma_start(
                            out=ms[32 * g:32 * (g + 1), h, g:31 + g, :],
                            in_=bass.AP(tensor=uwsb.tensor,
                                        offset=uwsb.offset + h * 1024 + 31 * 32,
                                        ap=[uwsb.ap[0], [-32, 31], [1, 32]]),
                        )
            # ---------------- weights ----------------------------------
            wq_sb = constp.tile([128, 2, 256], MDT)
            wk_sb = constp.tile([128, 2, 256], MDT)
            wv_sb = constp.tile([128, 2, 512], MDT)
            wo_sb = constp.tile([128, 4, 256], MDT)
            for dst_sb, wsrc, nk in ((wq_sb, wq, 2), (wk_sb, wk, 2),
                                     (wv_sb, wv, 2), (wo_sb, wo, 4)):
                for kt in range(nk):
                    wstg = xinp.tile([128, 512], F32, tag="wstg", bufs=2)
                    cols = dst_sb.shape[2]
                    nc.scalar.dma_start(out=wstg[:, 0:cols],
                                          in_=wsrc[128 * kt:128 * (kt + 1), :])
                    nc.vector.tensor_copy(dst_sb[:, kt, :], wstg[:, 0:cols])

            # BN affine rows (partition-broadcast straight from DRAM):
            # g2 = gamma*c ; b2 = bo*g2 + beta
            g2b = constp.tile([128, DOUT], F32)
            b2b = constp.tile([128, DOUT], F32)
            tmpb = constp.tile([128, DOUT], F32)
            nc.sync.dma_start(
                out=g2b, in_=bass.AP(tensor=gam.tensor, offset=0,
                                     ap=[[0, 128], [1, DOUT]]))
            nc.sync.dma_start(
                out=b2b, in_=bass.AP(tensor=bet.tensor, offset=0,
                                     ap=[[0, 128], [1, DOUT]]))
            nc.sync.dma_start(
                out=tmpb, in_=bass.AP(tensor=bo.tensor, offset=0,
                                      ap=[[0, 128], [1, DOUT]]))
            nc.scalar.mul(g2b, g2b, BN_C)
            nc.vector.tensor_mul(tmpb, tmpb, g2b)
            nc.vector.tensor_add(b2b, b2b, tmpb)

            # ---------------- phase A: x -> xT --------------------------
            xT = bigp.tile([128, 2, N], MDT)
            for nt in range(8):
                xa = xinp.tile([128, D], F32, tag="xa", bufs=3)
                nc.scalar.dma_start(out=xa, in_=x[128 * nt:128 * (nt + 1), :])
                for dt in range(2):
                    pst = ps1p.tile([128, 128], F32, tag="ps1")
                    nc.tensor.transpose(pst, xa[:, 128 * dt:128 * (dt + 1)], ident)
                    nc.vector.tensor_copy(xT[:, dt, 128 * nt:128 * (nt + 1)], pst)

            # ---------------- phase B: QKV proj -------------------------
            qT = bigp.tile([128, 2, N], MDT)
            kT = bigp.tile([128, 2, N], MDT)
            for dst_sb, w_sb in ((qT, wq_sb), (kT, wk_sb)):
                for mt in range(2):
                    for ic in range(2):
                        ps = ps1p.tile([128, 512], F32, tag="ps1")
                        for kt in range(2):
                            nc.tensor.matmul(
                                ps,
                                _r(w_sb[:, kt, 128 * mt:128 * (mt + 1)]),
                                _r(xT[:, kt, 512 * ic:512 * (ic + 1)]),
                                start=(kt == 0), stop=(kt == 1),
                            )
                        nc.vector.tensor_copy(dst_sb[:, mt, 512 * ic:512 * (ic + 1)], ps)

            # v, augmented with a ones column per head for the softmax
            # denominator: lhsT = [v | 1] -> denominator lands at out row 64.
            va = bigp.tile([128, 8, 8, 65], MDT)
            # ones columns only: ACT Copy with scale=0, bias=1 writes 1.0
            # into the 65th column of each head slot (the v copies fill the
            # rest); avoids a 40us whole-tile memset on the gpsimd engine
            nc.scalar.activation(va[:, :, :, 64:65],
                                 ident[:, 0:64],
                                 mybir.ActivationFunctionType.Copy,
                                 bias=1.0, scale=0.0)
            for jt in range(8):
                ps = ps1p.tile([128, 512], F32, tag="ps1")
                for kt in range(2):
                    nc.tensor.matmul(
                        ps,
                        _r(xT[:, kt, 128 * jt:128 * (jt + 1)]),
                        _r(wv_sb[:, kt, :]),
                        start=(kt == 0), stop=(kt == 1),
                    )
                psr = ps.rearrange("p (h v) -> p h v", v=64)
                nc.vector.tensor_copy(va[:, jt, :, 0:64], psr)

            # ---------------- phase C: attention ------------------------
            gT = bigp.tile([128, 4, N], MDT)
            for h in range(H):
                mtk = h // 4
                pb = 32 * (h % 4)
                po = ps1p.tile([128, 1024], F32, tag="ps1")
                es_prev = None
                for jt in range(9):
                    if jt < 8:
                        ps = ps2p.tile([128, 1024], F32, tag="st")
                        for ic in range(2):
                            nc.tensor.matmul(
                                ps[:, 512 * ic:512 * (ic + 1)],
                                _r(kT[pb:pb + 32, mtk, 128 * jt:128 * (jt + 1)]),
                                _r(qT[pb:pb + 32, mtk, 512 * ic:512 * (ic + 1)]),
                                start=True, stop=False,
                                tile_position=(pb, 0),
                            )
                            u0 = 16 * ic + 31 - 4 * jt
                            nc.tensor.matmul(
                                ps[:, 512 * ic:512 * (ic + 1)],
                                _r(isc),
                                ms[:, h, u0:u0 + 16, :],
                                start=False, stop=True,
                            )
                        es = expp.tile([128, 1024], MDT, tag="es")
                        nc.scalar.activation(es, ps,
                                             mybir.ActivationFunctionType.Exp,
                                             scale=SCALE)
                    # attnV one stage behind so PE never stalls on exp
                    if jt > 0:
                        for ic in range(2):
                            nc.tensor.matmul(
                                po[0:65, 512 * ic:512 * (ic + 1)],
                                _r(va[:, jt - 1, h, :]),
                                _r(es_prev[:, 512 * ic:512 * (ic + 1)]),
                                start=(jt == 1), stop=(jt == 8),
                            )
                    es_prev = es
                # normalize: out/denominator (DMA-broadcast the recip row
                # across partitions; engines cannot step-0 broadcast)
                rr = smallp.tile([1, N], F32, tag="rr", bufs=1)
                nc.vector.reciprocal(rr, po[64:65, :])
                nc.sync.dma_start(out=rrd[h, :].unsqueeze(0), in_=rr)
                rrb = smallp.tile([64, N], F32, tag="rrb", bufs=1)
                nc.sync.dma_start(
                    out=rrb,
                    in_=bass.AP(tensor=rrd.tensor, offset=h * N,
                                ap=[[0, 64], [1, N]]),
                )
                nc.vector.tensor_mul(
                    gT[64 * (h % 2):64 * (h % 2) + 64, h // 2, :],
                    po[0:64, :],
                    rrb,
                )

            # ---------------- phase D: gelu (one table switch, in-place) -
            for kt in range(4):
                nc.scalar.activation(gT[:, kt, :], gT[:, kt, :],
                                     mybir.ActivationFunctionType.Gelu)

            # ---------------- phase E: out proj + BN --------------------
            for it in range(8):
                ps = ps1p.tile([128, 512], F32, tag="ps1")
                for kt in range(4):
                    nc.tensor.matmul(
                        ps[:, 0:256],
                        _r(gT[:, kt, 128 * it:128 * (it + 1)]),
                        _r(wo_sb[:, kt, :]),
                        start=(kt == 0), stop=(kt == 3),
                    )
                yt = youtp.tile([128, DOUT], F32, tag="yt")
                nc.vector.tensor_mul(yt, ps[:, 0:256], g2b)
                nc.vector.tensor_add(yt, yt, b2b)
                nc.sync.dma_start(out=out[128 * it:128 * (it + 1), :], in_=yt)

    _split_excess_waits(nc)
    return nc


def _split_excess_waits(nc):
    """walrus rejects >1 sem-wait per instruction ("Too many sync wait
    commands"); unroll extras into a chain of single-wait same-engine
    NoOps directly before the instruction."""
    ctr = 0
    for fn in nc.m.functions:
        for blk in fn.blocks:
            out = []
            for inst in blk.instructions:
                si = inst.sync_info
                if si is not None and len(si.on_wait) > 1:
                    for w in si.on_wait[:-1]:
                        nop = mybir.InstNoOp(name=f"waitnop-{ctr}")
                        ctr += 1
                        nop.engine = inst.engine
                        nop.sync_info = mybir.SyncInfo(
                            on_wait=[w], on_update=[])
                        out.append(nop)
                    inst.sync_info = mybir.SyncInfo(
                        on_wait=[si.on_wait[-1]], on_update=list(si.on_update))
                out.append(inst)
            blk.instructions = out


_NC_CACHE = None


def kernel(**inputs) -> np.ndarray:
    global _NC_CACHE
    x = np.ascontiguousarray(inputs["x"], dtype=np.float32)        # (8,32,32,256)
    shared = {
        "wq": np.ascontiguousarray(inputs["Wq"], dtype=np.float32),
        "wk": np.ascontiguousarray(inputs["Wk"], dtype=np.float32),
        "wv": np.ascontiguousarray(inputs["Wv"], dtype=np.float32),
        "wo": np.ascontiguousarray(inputs["Wo"], dtype=np.float32),
        "pe": np.ascontiguousarray(inputs["pos_emb"], dtype=np.float32),
        "bo": np.ascontiguousarray(inputs["bo"], dtype=np.float32),
        "gam": np.ascontiguousarray(inputs["gamma"], dtype=np.float32),
        "bet": np.ascontiguousarray(inputs["beta"], dtype=np.float32),
    }
    in_maps = []
    for c in range(NCORES):
        m = dict(shared)
        m["x"] = np.ascontiguousarray(x[c].reshape(N, D))
        in_maps.append(m)

    if _NC_CACHE is None:
        _NC_CACHE = build_nc()
    res = run_bass_kernel_spmd(_NC_CACHE, in_maps, core_ids=list(range(NCORES)))
    outs = [res.results[c]["out"].reshape(FM, FM, DOUT) for c in range(NCORES)]
    return np.stack(outs, axis=0)


if __name__ == "__main__":
    build_nc()
    print("build ok")



# revision 7
# speedup vs baseline: 1.1336x; 1.1336x over previous
"""Trainium2 Bass kernel for nn_Attention_89172110999574.

Strategy (8 NeuronCores, data parallel — 1 batch element per core):
  - x transposed on-chip via PE; QKV projections as matmuls (fp32r).
  - Scores computed TRANSPOSED (ST[j,i] = k_j . q_i) so that softmax
    reduction rides the attn@V matmul: rhs is exp(ST), lhsT is [v | 1],
    giving the softmax denominator as an extra output row for free.
  - Relative-position bias: the bias matrix is block-Toeplitz, so a
    per-head strip table MS[(g,cj), h, u, ci] = 32*E_h[|u-31-g|, |ci-cj|]
    is precomputed ON HOST and shipped as a DRAM input in fp8e4m3 as a
    (main, residual) pair; every score tile's bias is then added into
    PSUM by ONE DoubleRow identity matmul (0.5 cycles/row) that sums
    main+residual — half the PE cost of the fp32 identity add, ~exact.
  - exp() without max-subtraction (scores are ~N(0,1); safe in fp32),
    gelu deferred to a single phase to avoid ACT table-set switches.
  - fp32r (full-rate fp32 PE mode) for all big matmuls.
"""

import os
import sys

import numpy as np

for _p in ("/opt/trn_rl_repo", "/root/.axon_site/_ro/trn_rl_repo"):
    if os.path.isdir(_p) and _p not in sys.path:
        sys.path.insert(0, _p)

import concourse.bass as bass
import concourse.tile as tile
from concourse import mybir
from concourse.bass_utils import run_bass_kernel_spmd
from concourse.masks import make_identity

N = 1024          # tokens per batch (32*32)
D = 256           # model dim
H = 8             # heads
DK = 32           # head dim (qk)
DV = 64           # head dim (v)
DOUT = 256        # output dim
NCORES = 8
FM = 32           # fmap
SCALE = float(DK) ** -0.5          # 1/sqrt(32)
BN_C = float(1.0 / np.sqrt(1.0 + 1e-5))
F32 = mybir.dt.float32
F32R = mybir.dt.float32r
F8 = mybir.dt.float8e4

MDT = F32R

# ms strip table geometry (free-dim element strides inside the SBUF tile)
MS_U = 66
MS_HSTR = MS_U * 32          # 2112 elements per head
MS_TSTR = H * MS_HSTR        # 16896 elements per (main|residual) plane


def build_nc():
    nc = bass.Bass("TRN2", target_bir_lowering=False, debug=False)

    x = nc.dram_tensor("x", [N, D], F32, kind="ExternalInput").ap()
    wq = nc.dram_tensor("wq", [D, H * DK], F32R, kind="ExternalInput").ap()
    wk = nc.dram_tensor("wk", [D, H * DK], F32R, kind="ExternalInput").ap()
    wv = nc.dram_tensor("wv", [D, H * DV], F32R, kind="ExternalInput").ap()
    wo = nc.dram_tensor("wo", [H * DV, DOUT], F32R, kind="ExternalInput").ap()
    ms8d = nc.dram_tensor("ms8", [128, 2, H, MS_U, 32], F8,
                          kind="ExternalInput").ap()
    bo = nc.dram_tensor("bo", [DOUT], F32, kind="ExternalInput").ap()
    gam = nc.dram_tensor("gam", [DOUT], F32, kind="ExternalInput").ap()
    bet = nc.dram_tensor("bet", [DOUT], F32, kind="ExternalInput").ap()
    out = nc.dram_tensor("out", [N, DOUT], F32, kind="ExternalOutput").ap()

    # scratch DRAM for partition-broadcasting the softmax recip rows
    rrd = nc.dram_tensor("rrd", [H, N], F32).ap()

    with tile.TileContext(nc) as tc:
        with (
            tc.tile_pool(name="const", bufs=1) as constp,
            tc.tile_pool(name="big", bufs=1) as bigp,
            tc.tile_pool(name="xin", bufs=3) as xinp,
            tc.tile_pool(name="exps", bufs=3) as expp,
            tc.tile_pool(name="small", bufs=2) as smallp,
            tc.tile_pool(name="yout", bufs=3) as youtp,
            tc.tile_pool(name="ps1", bufs=2, space="PSUM") as ps1p,
            tc.tile_pool(name="ps2", bufs=2, space="PSUM") as ps2p,
        ):
            # ---------------- weights (direct DMA, f32r bits == f32) ----
            wq_sb = constp.tile([128, 2, 256], MDT)
            wk_sb = constp.tile([128, 2, 256], MDT)
            wv_sb = constp.tile([128, 2, 512], MDT)
            wo_sb = constp.tile([128, 4, 256], MDT)
            for dst_sb, wsrc, nk in ((wq_sb, wq, 2), (wk_sb, wk, 2),
                                     (wv_sb, wv, 2), (wo_sb, wo, 4)):
                for kt in range(nk):
                    eng = nc.scalar if kt % 2 == 0 else nc.sync
                    eng.dma_start(out=dst_sb[:, kt, :],
                                  in_=wsrc[128 * kt:128 * (kt + 1), :])

            # ---------------- bias strip table (host-precomputed) -------
            ms8 = constp.tile([128, 2, H, MS_U, 32], F8)
            for hp in range(4):
                eng = nc.sync if hp % 2 == 0 else nc.gpsimd
                eng.dma_start(out=ms8[:, :, 2 * hp:2 * hp + 2],
                              in_=ms8d[:, :, 2 * hp:2 * hp + 2])

            # ---------------- constants ---------------------------------
            ident = constp.tile([128, 128], F32)
            make_identity(nc, ident)
            id8 = constp.tile([128, 128], F8)
            nc.vector.tensor_copy(id8, ident)

            # BN affine rows (partition-broadcast straight from DRAM):
            # g2 = gamma*c ; b2 = bo*g2 + beta
            g2b = constp.tile([128, DOUT], F32)
            b2b = constp.tile([128, DOUT], F32)
            tmpb = constp.tile([128, DOUT], F32)
            nc.sync.dma_start(
                out=g2b, in_=bass.AP(tensor=gam.tensor, offset=0,
                                     ap=[[0, 128], [1, DOUT]]))
            nc.sync.dma_start(
                out=b2b, in_=bass.AP(tensor=bet.tensor, offset=0,
                                     ap=[[0, 128], [1, DOUT]]))
            nc.sync.dma_start(
                out=tmpb, in_=bass.AP(tensor=bo.tensor, offset=0,
                                      ap=[[0, 128], [1, DOUT]]))
            nc.scalar.mul(g2b, g2b, BN_C)
            nc.vector.tensor_mul(tmpb, tmpb, g2b)
            nc.vector.tensor_add(b2b, b2b, tmpb)

            # ---------------- phase A: x -> xT --------------------------
            xT = bigp.tile([128, 2, N], MDT)
            for nt in range(8):
                xa = xinp.tile([128, D], F32, tag="xa", bufs=3)
                nc.scalar.dma_start(out=xa, in_=x[128 * nt:128 * (nt + 1), :])
                for dt in range(2):
                    pst = ps1p.tile([128, 128], F32, tag="ps1")
                    nc.tensor.transpose(pst, xa[:, 128 * dt:128 * (dt + 1)], ident)
                    nc.vector.tensor_copy(xT[:, dt, 128 * nt:128 * (nt + 1)], pst)

            # ---------------- phase B: QKV proj -------------------------
            qT = bigp.tile([128, 2, N], MDT)
            kT = bigp.tile([128, 2, N], MDT)
            for dst_sb, w_sb in ((qT, wq_sb), (kT, wk_sb)):
                for mt in range(2):
                    for ic in range(2):
                        ps = ps1p.tile([128, 512], F32, tag="ps1")
                        for kt in range(2):
                            nc.tensor.matmul(
                                ps,
                                w_sb[:, kt, 128 * mt:128 * (mt + 1)],
                                xT[:, kt, 512 * ic:512 * (ic + 1)],
                                start=(kt == 0), stop=(kt == 1),
                            )
                        nc.vector.tensor_copy(dst_sb[:, mt, 512 * ic:512 * (ic + 1)], ps)

            # v, augmented with a ones column per head for the softmax
            # denominator: lhsT = [v | 1] -> denominator lands at out row 64.
            va = bigp.tile([128, 8, 8, 65], MDT)
            nc.scalar.activation(va[:, :, :, 64:65],
                                 ident[:, 0:64],
                                 mybir.ActivationFunctionType.Copy,
                                 bias=1.0, scale=0.0)
            for jt in range(8):
                ps = ps1p.tile([128, 512], F32, tag="ps1")
                for kt in range(2):
                    nc.tensor.matmul(
                        ps,
                        xT[:, kt, 128 * jt:128 * (jt + 1)],
                        wv_sb[:, kt, :],
                        start=(kt == 0), stop=(kt == 1),
                    )
                psr = ps.rearrange("p (h v) -> p h v", v=64)
                nc.vector.tensor_copy(va[:, jt, :, 0:64], psr)

            # ---------------- phase C: attention ------------------------
            # per (h, jt, ic): kq matmul (512c) + DoubleRow bias identity
            # matmul (256c) into the same PSUM region; exp on ACT; attnV
            # one stage behind on PE.
            id8_pair = bass.AP(tensor=id8.tensor, offset=id8.offset,
                               ap=[id8.ap[0], [0, 2], [1, 128]])
            gT = bigp.tile([128, 4, N], MDT)
            for h in range(H):
                mtk = h // 4
                pb = 32 * (h % 4)
                po = ps1p.tile([128, 1024], F32, tag="ps1")
                es_prev = None
                for jt in range(9):
                    if jt < 8:
                        ps = ps2p.tile([128, 1024], F32, tag="st")
                        for ic in range(2):
                            nc.tensor.matmul(
                                ps[:, 512 * ic:512 * (ic + 1)],
                                kT[pb:pb + 32, mtk, 128 * jt:128 * (jt + 1)],
                                qT[pb:pb + 32, mtk, 512 * ic:512 * (ic + 1)],
                                start=True, stop=False,
                                tile_position=(pb, 0),
                            )
                            u0 = 16 * ic + 31 - 4 * jt
                            nc.tensor.matmul(
                                ps[:, 512 * ic:512 * (ic + 1)],
                                id8_pair,
                                bass.AP(tensor=ms8.tensor,
                                        offset=(ms8.offset + h * MS_HSTR
                                                + u0 * 32),
                                        ap=[ms8.ap[0], [MS_TSTR, 2], [1, 512]]),
                                start=False, stop=True,
                                perf_mode=mybir.MatmulPerfMode.DoubleRow,
                            )
                        es = expp.tile([128, 1024], MDT, tag="es")
                        nc.scalar.activation(es, ps,
                                             mybir.ActivationFunctionType.Exp,
                                             scale=SCALE)
                    # attnV one stage behind so PE never stalls on exp
                    if jt > 0:
                        for ic in range(2):
                            nc.tensor.matmul(
                                po[0:65, 512 * ic:512 * (ic + 1)],
                                va[:, jt - 1, h, :],
                                es_prev[:, 512 * ic:512 * (ic + 1)],
                                start=(jt == 1), stop=(jt == 8),
                            )
                    es_prev = es
                # normalize: out/denominator (DMA-broadcast the recip row
                # across partitions; engines cannot step-0 broadcast)
                rr = smallp.tile([1, N], F32, tag="rr", bufs=1)
                nc.vector.reciprocal(rr, po[64:65, :])
                nc.sync.dma_start(out=rrd[h, :].unsqueeze(0), in_=rr)
                rrb = smallp.tile([64, N], F32, tag="rrb", bufs=1)
                nc.sync.dma_start(
                    out=rrb,
                    in_=bass.AP(tensor=rrd.tensor, offset=h * N,
                                ap=[[0, 64], [1, N]]),
                )
                nc.vector.tensor_mul(
                    gT[64 * (h % 2):64 * (h % 2) + 64, h // 2, :],
                    po[0:64, :],
                    rrb,
                )

            # ---------------- phase D: gelu (one table switch, in-place) -
            for kt in range(4):
                nc.scalar.activation(gT[:, kt, :], gT[:, kt, :],
                                     mybir.ActivationFunctionType.Gelu)

            # ---------------- phase E: out proj + BN --------------------
            for it in range(8):
                ps = ps1p.tile([128, 512], F32, tag="ps1")
                for kt in range(4):
                    nc.tensor.matmul(
                        ps[:, 0:256],
                        gT[:, kt, 128 * it:128 * (it + 1)],
                        wo_sb[:, kt, :],
                        start=(kt == 0), stop=(kt == 3),
                    )
                yt = youtp.tile([128, DOUT], F32, tag="yt")
                nc.vector.tensor_mul(yt, ps[:, 0:256], g2b)
                nc.vector.tensor_add(yt, yt, b2b)
                nc.sync.dma_start(out=out[128 * it:128 * (it + 1), :], in_=yt)

    _split_excess_waits(nc)
    return nc


def _split_excess_waits(nc):
    """walrus rejects >1 sem-wait per instruction ("Too many sync wait
    commands"); unroll extras into a chain of single-wait same-engine
    NoOps directly before the instruction."""
    ctr = 0
    for fn in nc.m.functions:
        for blk in fn.blocks:
            out = []
            for inst in blk.instructions:
                si = inst.sync_info
                if si is not None and len(si.on_wait) > 1:
                    for w in si.on_wait[:-1]:
                        nop = mybir.InstNoOp(name=f"waitnop-{ctr}")
                        ctr += 1
                        nop.engine = inst.engine
                        nop.sync_info = mybir.SyncInfo(
                            on_wait=[w], on_update=[])
                        out.append(nop)
                    inst.sync_info = mybir.SyncInfo(
                        on_wait=[si.on_wait[-1]], on_update=list(si.on_update))
                out.append(inst)
            blk.instructions = out


def _build_ms8(pos_emb: np.ndarray) -> np.ndarray:
    """Host-precompute the fp8 (main, residual) bias strip table.

    table[(g,cj), t, h, u, ci] approximates 32*E_h[|u-31-g|, |ci-cj|]
    (main + residual), where E = pos_emb.reshape(32, 32, H).
    """
    import ml_dtypes

    E = np.asarray(pos_emb, dtype=np.float32).reshape(32, 32, H)
    T = E.transpose(2, 0, 1)                                   # [h, a, b]
    g = np.arange(4)
    u = np.arange(MS_U)
    a_idx = np.abs(u[None, :] - 31 - g[:, None]).clip(0, 31)   # [4, 66]
    c = np.arange(32)
    b_idx = np.abs(c[None, :] - c[:, None])                    # [cj, ci]
    tmp = T[:, a_idx]                                          # [h, 4, 66, b]
    tab = tmp[:, :, :, b_idx]                                  # [h, 4, 66, cj, ci]
    # -> [(g, cj), h, u, ci]
    arr = np.ascontiguousarray(tab.transpose(1, 3, 0, 2, 4)).reshape(
        4 * 32, H, MS_U, 32) * np.float32(DK)
    main = arr.astype(ml_dtypes.float8_e4m3)
    res = (arr - main.astype(np.float32)).astype(ml_dtypes.float8_e4m3)
    return np.ascontiguousarray(
        np.stack([main, res], axis=1))                         # [128, 2, h, u, ci]


_NC_CACHE = None


def kernel(**inputs) -> np.ndarray:
    global _NC_CACHE
    x = np.ascontiguousarray(inputs["x"], dtype=np.float32)        # (8,32,32,256)
    shared = {
        "wq": np.ascontiguousarray(inputs["Wq"], dtype=np.float32),
        "wk": np.ascontiguousarray(inputs["Wk"], dtype=np.float32),
        "wv": np.ascontiguousarray(inputs["Wv"], dtype=np.float32),
        "wo": np.ascontiguousarray(inputs["Wo"], dtype=np.float32),
        "ms8": _build_ms8(inputs["pos_emb"]),
        "bo": np.ascontiguousarray(inputs["bo"], dtype=np.float32),
        "gam": np.ascontiguousarray(inputs["gamma"], dtype=np.float32),
        "bet": np.ascontiguousarray(inputs["beta"], dtype=np.float32),
    }
    in_maps = []
    for c in range(NCORES):
        m = dict(shared)
        m["x"] = np.ascontiguousarray(x[c].reshape(N, D))
        in_maps.append(m)

    if _NC_CACHE is None:
        _NC_CACHE = build_nc()
    res = run_bass_kernel_spmd(_NC_CACHE, in_maps, core_ids=list(range(NCORES)))
    outs = [res.results[c]["out"].reshape(FM, FM, DOUT) for c in range(NCORES)]
    return np.stack(outs, axis=0)


if __name__ == "__main__":
    build_nc()
    print("build ok")


# revision 12
# speedup vs baseline: 1.2236x; 1.0794x over previous
"""Trainium2 Bass kernel for nn_Attention_89172110999574.

Strategy (8 NeuronCores, data parallel — 1 batch element per core):
  - x transposed on-chip via PE; QKV projections as matmuls (fp32r).
  - Scores computed TRANSPOSED (ST[j,i] = k_j . q_i) so that softmax
    reduction rides the attn@V matmul: rhs is exp(ST), lhsT is [v | 1],
    giving the softmax denominator as an extra output row for free.
  - Relative-position bias: the bias matrix is block-Toeplitz, so a
    per-head strip table MS[(g,cj), h, u, ci] = 32*E_h[|u-31-g|, |ci-cj|]
    is precomputed ON HOST and shipped as a DRAM input in fp8e4m3 as a
    (main, residual) pair; every score tile's bias is then added into
    PSUM by ONE DoubleRow identity matmul (0.5 cycles/row) that sums
    main+residual — half the PE cost of the fp32 identity add, ~exact.
  - exp() without max-subtraction (scores are ~N(0,1); safe in fp32),
    gelu deferred to a single phase to avoid ACT table-set switches.
  - fp32r (full-rate fp32 PE mode) for all big matmuls.
"""

import os
import sys

import numpy as np

for _p in ("/opt/trn_rl_repo", "/root/.axon_site/_ro/trn_rl_repo"):
    if os.path.isdir(_p) and _p not in sys.path:
        sys.path.insert(0, _p)

import concourse.bass as bass
import concourse.tile as tile
from concourse import mybir
from concourse.bass_utils import run_bass_kernel_spmd
from concourse.masks import make_identity

N = 1024          # tokens per batch (32*32)
D = 256           # model dim
H = 8             # heads
DK = 32           # head dim (qk)
DV = 64           # head dim (v)
DOUT = 256        # output dim
NCORES = 8
FM = 32           # fmap
SCALE = float(DK) ** -0.5          # 1/sqrt(32)
BN_C = float(1.0 / np.sqrt(1.0 + 1e-5))
F32 = mybir.dt.float32
F32R = mybir.dt.float32r
F8 = mybir.dt.float8e4

MDT = F32R

# ms strip table geometry (free-dim element strides inside the SBUF tile)
MS_U = 66
MS_HSTR = MS_U * 32          # 2112 elements per head
MS_TSTR = H * MS_HSTR        # 16896 elements per (main|residual) plane


def build_nc():
    nc = bass.Bass("TRN2", target_bir_lowering=False, debug=False)

    x = nc.dram_tensor("x", [N, D], F32, kind="ExternalInput").ap()
    wq = nc.dram_tensor("wq", [D, H * DK], F32R, kind="ExternalInput").ap()
    wk = nc.dram_tensor("wk", [D, H * DK], F32R, kind="ExternalInput").ap()
    wv = nc.dram_tensor("wv", [D, H * DV], F32R, kind="ExternalInput").ap()
    wo = nc.dram_tensor("wo", [H * DV, DOUT], F32R, kind="ExternalInput").ap()
    ms8d = nc.dram_tensor("ms8", [128, 2, H, MS_U, 32], F8,
                          kind="ExternalInput").ap()
    bo = nc.dram_tensor("bo", [DOUT], F32, kind="ExternalInput").ap()
    gam = nc.dram_tensor("gam", [DOUT], F32, kind="ExternalInput").ap()
    bet = nc.dram_tensor("bet", [DOUT], F32, kind="ExternalInput").ap()
    out = nc.dram_tensor("out", [N, DOUT], F32, kind="ExternalOutput").ap()

    # scratch DRAM for partition-broadcasting the softmax recip rows
    rrd = nc.dram_tensor("rrd", [H, N], F32).ap()

    with tile.TileContext(nc) as tc:
        with (
            tc.tile_pool(name="const", bufs=1) as constp,
            tc.tile_pool(name="big", bufs=1) as bigp,
            tc.tile_pool(name="xin", bufs=3) as xinp,
            tc.tile_pool(name="exps", bufs=3) as expp,
            tc.tile_pool(name="small", bufs=2) as smallp,
            tc.tile_pool(name="yout", bufs=3) as youtp,
            tc.tile_pool(name="ps1", bufs=2, space="PSUM") as ps1p,
            tc.tile_pool(name="ps2", bufs=2, space="PSUM") as ps2p,
        ):
            # ---------------- phase A: x -> xT --------------------------
            # x tiles are DMA'd first: the serial DMA device must deliver
            # them before anything else so PE can start transposing.
            ident = constp.tile([128, 128], F32)
            make_identity(nc, ident)
            xT = bigp.tile([128, 2, N], MDT)
            xas = []
            for nt in range(8):
                xa = xinp.tile([128, D], F32, tag="xa", bufs=8)
                nc.scalar.dma_start(out=xa, in_=x[128 * nt:128 * (nt + 1), :])
                xas.append(xa)

            # weights for q/k next on the wire, then the first bias chunk
            wq_sb = constp.tile([128, 2, 256], MDT)
            wk_sb = constp.tile([128, 2, 256], MDT)
            wv_sb = constp.tile([128, 2, 512], MDT)
            wo_sb = constp.tile([128, 4, 256], MDT)
            for kt in range(2):
                nc.sync.dma_start(out=wq_sb[:, kt, :],
                                  in_=wq[128 * kt:128 * (kt + 1), :])
                nc.sync.dma_start(out=wk_sb[:, kt, :],
                                  in_=wk[128 * kt:128 * (kt + 1), :])

            ms8 = constp.tile([128, 2, H, MS_U, 32], F8)
            nc.gpsimd.dma_start(out=ms8[:, :, 0:1], in_=ms8d[:, :, 0:1])

            for nt in range(8):
                for dt in range(2):
                    pst = ps1p.tile([128, 128], F32, tag="ps1")
                    nc.tensor.transpose(pst, xas[nt][:, 128 * dt:128 * (dt + 1)],
                                        ident)
                    nc.vector.tensor_copy(xT[:, dt, 128 * nt:128 * (nt + 1)], pst)

            for kt in range(2):
                nc.sync.dma_start(out=wv_sb[:, kt, :],
                                  in_=wv[128 * kt:128 * (kt + 1), :])
            nc.gpsimd.dma_start(out=ms8[:, :, 1:2], in_=ms8d[:, :, 1:2])

            id8 = constp.tile([128, 128], F8)
            nc.vector.tensor_copy(id8, ident)

            # ---------------- phase B: QKV proj -------------------------
            qT = bigp.tile([128, 2, N], MDT)
            kT = bigp.tile([128, 2, N], MDT)
            for ic in range(2):
                for dst_sb, w_sb in ((qT, wq_sb), (kT, wk_sb)):
                    for mt in range(2):
                        ps = ps1p.tile([128, 512], F32, tag="ps1")
                        for kt in range(2):
                            nc.tensor.matmul(
                                ps,
                                w_sb[:, kt, 128 * mt:128 * (mt + 1)],
                                xT[:, kt, 512 * ic:512 * (ic + 1)],
                                start=(kt == 0), stop=(kt == 1),
                            )
                        nc.vector.tensor_copy(dst_sb[:, mt, 512 * ic:512 * (ic + 1)], ps)

            # v, augmented with a ones column per head for the softmax
            # denominator: lhsT = [v | 1] -> denominator lands at out row 64.
            va = bigp.tile([128, 8, 8, 65], MDT)
            nc.scalar.activation(va[:, :, :, 64:65],
                                 ident[:, 0:64],
                                 mybir.ActivationFunctionType.Copy,
                                 bias=1.0, scale=0.0)
            for jt in range(8):
                ps = ps1p.tile([128, 512], F32, tag="ps1")
                for kt in range(2):
                    nc.tensor.matmul(
                        ps,
                        xT[:, kt, 128 * jt:128 * (jt + 1)],
                        wv_sb[:, kt, :],
                        start=(kt == 0), stop=(kt == 1),
                    )
                psr = ps.rearrange("p (h v) -> p h v", v=64)
                nc.vector.tensor_copy(va[:, jt, :, 0:64], psr)

            # remaining bias chunks, wo, BN rows — needed progressively
            for hp in (1, 2, 3):
                eng = nc.sync if hp % 2 == 0 else nc.gpsimd
                eng.dma_start(out=ms8[:, :, 2 * hp:2 * hp + 2],
                              in_=ms8d[:, :, 2 * hp:2 * hp + 2])
            for kt in range(4):
                nc.scalar.dma_start(out=wo_sb[:, kt, :],
                                    in_=wo[128 * kt:128 * (kt + 1), :])

            # BN affine rows (partition-broadcast straight from DRAM):
            # g2 = gamma*c ; b2 = bo*g2 + beta
            g2b = constp.tile([128, DOUT], F32)
            b2b = constp.tile([128, DOUT], F32)
            tmpb = constp.tile([128, DOUT], F32)
            nc.scalar.dma_start(
                out=g2b, in_=bass.AP(tensor=gam.tensor, offset=0,
                                     ap=[[0, 128], [1, DOUT]]))
            nc.scalar.dma_start(
                out=b2b, in_=bass.AP(tensor=bet.tensor, offset=0,
                                     ap=[[0, 128], [1, DOUT]]))
            nc.scalar.dma_start(
                out=tmpb, in_=bass.AP(tensor=bo.tensor, offset=0,
                                      ap=[[0, 128], [1, DOUT]]))
            nc.scalar.mul(g2b, g2b, BN_C)
            nc.vector.tensor_mul(tmpb, tmpb, g2b)
            nc.vector.tensor_add(b2b, b2b, tmpb)

            # ---------------- phase C: attention ------------------------
            # per (h, jt, ic): kq matmul (512c) + DoubleRow bias identity
            # matmul (256c) into the same PSUM region; exp on ACT; attnV
            # one stage behind on PE.
            id8_pair = bass.AP(tensor=id8.tensor, offset=id8.offset,
                               ap=[id8.ap[0], [0, 2], [1, 128]])
            gT = bigp.tile([128, 4, N], MDT)
            for h in range(H):
                mtk = h // 4
                pb = 32 * (h % 4)
                po = ps1p.tile([128, 1024], F32, tag="ps1")
                es_prev = None
                for jt in range(9):
                    if jt < 8:
                        ps = ps2p.tile([128, 1024], F32, tag="st")
                        for ic in range(2):
                            nc.tensor.matmul(
                                ps[:, 512 * ic:512 * (ic + 1)],
                                kT[pb:pb + 32, mtk, 128 * jt:128 * (jt + 1)],
                                qT[pb:pb + 32, mtk, 512 * ic:512 * (ic + 1)],
                                start=True, stop=False,
                                tile_position=(pb, 0),
                            )
                            u0 = 16 * ic + 31 - 4 * jt
                            nc.tensor.matmul(
                                ps[:, 512 * ic:512 * (ic + 1)],
                                id8_pair,
                                bass.AP(tensor=ms8.tensor,
                                        offset=(ms8.offset + h * MS_HSTR
                                                + u0 * 32),
                                        ap=[ms8.ap[0], [MS_TSTR, 2], [1, 512]]),
                                start=False, stop=True,
                                perf_mode=mybir.MatmulPerfMode.DoubleRow,
                            )
                        es = expp.tile([128, 1024], MDT, tag="es")
                        nc.scalar.activation(es, ps,
                                             mybir.ActivationFunctionType.Exp,
                                             scale=SCALE)
                    # attnV one stage behind so PE never stalls on exp
                    if jt > 0:
                        for ic in range(2):
                            nc.tensor.matmul(
                                po[0:65, 512 * ic:512 * (ic + 1)],
                                va[:, jt - 1, h, :],
                                es_prev[:, 512 * ic:512 * (ic + 1)],
                                start=(jt == 1), stop=(jt == 8),
                            )
                    es_prev = es
                # normalize: out/denominator (DMA-broadcast the recip row
                # across partitions via DRAM; SBUF APs need nonzero
                # partition step)
                rr = smallp.tile([1, N], F32, tag="rr", bufs=1)
                nc.vector.reciprocal(rr, po[64:65, :])
                nc.sync.dma_start(out=rrd[h, :].unsqueeze(0), in_=rr)
                rrb = smallp.tile([64, N], F32, tag="rrb", bufs=1)
                nc.sync.dma_start(
                    out=rrb,
                    in_=bass.AP(tensor=rrd.tensor, offset=h * N,
                                ap=[[0, 64], [1, N]]),
                )
                nc.vector.tensor_mul(
                    gT[64 * (h % 2):64 * (h % 2) + 64, h // 2, :],
                    po[0:64, :],
                    rrb,
                )

            # ------- phase D/E: gelu + out proj + BN, pipelined ---------
            # gelu is applied per 128-column block (all 4 kt chunks of that
            # block in one ACT op) so each out-proj tile can start right
            # after its own gelu, overlapping ACT and PE in the tail.
            for it in range(8):
                gsl = bass.AP(tensor=gT.tensor,
                              offset=gT.offset + 128 * it,
                              ap=[gT.ap[0], [N, 4], [1, 128]])
                nc.scalar.activation(gsl, gsl,
                                     mybir.ActivationFunctionType.Gelu)
                ps = ps1p.tile([128, 512], F32, tag="ps1")
                for kt in range(4):
                    nc.tensor.matmul(
                        ps[:, 0:256],
                        gT[:, kt, 128 * it:128 * (it + 1)],
                        wo_sb[:, kt, :],
                        start=(kt == 0), stop=(kt == 3),
                    )
                yt = youtp.tile([128, DOUT], F32, tag="yt")
                nc.vector.tensor_mul(yt, ps[:, 0:256], g2b)
                nc.vector.tensor_add(yt, yt, b2b)
                nc.sync.dma_start(out=out[128 * it:128 * (it + 1), :], in_=yt)

    _split_excess_waits(nc)
    return nc


def _split_excess_waits(nc):
    """walrus rejects >1 sem-wait per instruction ("Too many sync wait
    commands"); unroll extras into a chain of single-wait same-engine
    NoOps directly before the instruction."""
    ctr = 0
    for fn in nc.m.functions:
        for blk in fn.blocks:
            out = []
            for inst in blk.instructions:
                si = inst.sync_info
                if si is not None and len(si.on_wait) > 1:
                    for w in si.on_wait[:-1]:
                        nop = mybir.InstNoOp(name=f"waitnop-{ctr}")
                        ctr += 1
                        nop.engine = inst.engine
                        nop.sync_info = mybir.SyncInfo(
                            on_wait=[w], on_update=[])
                        out.append(nop)
                    inst.sync_info = mybir.SyncInfo(
                        on_wait=[si.on_wait[-1]], on_update=list(si.on_update))
                out.append(inst)
            blk.instructions = out


def _build_ms8(pos_emb: np.ndarray) -> np.ndarray:
    """Host-precompute the fp8 (main, residual) bias strip table.

    table[(g,cj), t, h, u, ci] approximates 32*E_h[|u-31-g|, |ci-cj|]
    (main + residual), where E = pos_emb.reshape(32, 32, H).
    """
    import ml_dtypes

    E = np.asarray(pos_emb, dtype=np.float32).reshape(32, 32, H)
    T = E.transpose(2, 0, 1)                                   # [h, a, b]
    g = np.arange(4)
    u = np.arange(MS_U)
    a_idx = np.abs(u[None, :] - 31 - g[:, None]).clip(0, 31)   # [4, 66]
    c = np.arange(32)
    b_idx = np.abs(c[None, :] - c[:, None])                    # [cj, ci]
    tmp = T[:, a_idx]                                          # [h, 4, 66, b]
    tab = tmp[:, :, :, b_idx]                                  # [h, 4, 66, cj, ci]
    # -> [(g, cj), h, u, ci]
    arr = np.ascontiguousarray(tab.transpose(1, 3, 0, 2, 4)).reshape(
        4 * 32, H, MS_U, 32) * np.float32(DK)
    main = arr.astype(ml_dtypes.float8_e4m3)
    res = (arr - main.astype(np.float32)).astype(ml_dtypes.float8_e4m3)
    return np.ascontiguousarray(
        np.stack([main, res], axis=1))                         # [128, 2, h, u, ci]


_NC_CACHE = None


def kernel(**inputs) -> np.ndarray:
    global _NC_CACHE
    x = np.ascontiguousarray(inputs["x"], dtype=np.float32)        # (8,32,32,256)
    shared = {
        "wq": np.ascontiguousarray(inputs["Wq"], dtype=np.float32),
        "wk": np.ascontiguousarray(inputs["Wk"], dtype=np.float32),
        "wv": np.ascontiguousarray(inputs["Wv"], dtype=np.float32),
        "wo": np.ascontiguousarray(inputs["Wo"], dtype=np.float32),
        "ms8": _build_ms8(inputs["pos_emb"]),
        "bo": np.ascontiguousarray(inputs["bo"], dtype=np.float32),
        "gam": np.ascontiguousarray(inputs["gamma"], dtype=np.float32),
        "bet": np.ascontiguousarray(inputs["beta"], dtype=np.float32),
    }
    in_maps = []
    for c in range(NCORES):
        m = dict(shared)
        m["x"] = np.ascontiguousarray(x[c].reshape(N, D))
        in_maps.append(m)

    if _NC_CACHE is None:
        _NC_CACHE = build_nc()
    res = run_bass_kernel_spmd(_NC_CACHE, in_maps, core_ids=list(range(NCORES)))
    outs = [res.results[c]["out"].reshape(FM, FM, DOUT) for c in range(NCORES)]
    return np.stack(outs, axis=0)


if __name__ == "__main__":
    build_nc()
    print("build ok")


# revision 15
# speedup vs baseline: 1.2277x; 1.0033x over previous
"""Trainium2 Bass kernel for nn_Attention_89172110999574.

Strategy (8 NeuronCores, data parallel — 1 batch element per core):
  - x transposed on-chip via PE; QKV projections as matmuls (fp32r).
  - Scores computed TRANSPOSED (ST[j,i] = k_j . q_i) so that softmax
    reduction rides the attn@V matmul: rhs is exp(ST), lhsT is [v | 1],
    giving the softmax denominator as an extra output row for free.
  - Relative-position bias: the bias matrix is block-Toeplitz, so a
    per-head strip table MS[(g,cj), h, u, ci] = 32*E_h[|u-31-g|, |ci-cj|]
    is precomputed ON HOST and shipped as a DRAM input in fp8e4m3 as a
    (main, residual) pair; every score tile's bias is then added into
    PSUM by ONE DoubleRow identity matmul (0.5 cycles/row) that sums
    main+residual — half the PE cost of the fp32 identity add, ~exact.
  - exp() without max-subtraction (scores are ~N(0,1); safe in fp32),
    gelu deferred to a single phase to avoid ACT table-set switches.
  - fp32r (full-rate fp32 PE mode) for all big matmuls.
"""

import os
import sys

import numpy as np

for _p in ("/opt/trn_rl_repo", "/root/.axon_site/_ro/trn_rl_repo"):
    if os.path.isdir(_p) and _p not in sys.path:
        sys.path.insert(0, _p)

import concourse.bass as bass
import concourse.tile as tile
from concourse import mybir
from concourse.bass_utils import run_bass_kernel_spmd
from concourse.masks import make_identity

N = 1024          # tokens per batch (32*32)
D = 256           # model dim
H = 8             # heads
DK = 32           # head dim (qk)
DV = 64           # head dim (v)
DOUT = 256        # output dim
NCORES = 8
FM = 32           # fmap
SCALE = float(DK) ** -0.5          # 1/sqrt(32)
BN_C = float(1.0 / np.sqrt(1.0 + 1e-5))
F32 = mybir.dt.float32
F32R = mybir.dt.float32r
F8 = mybir.dt.float8e4

MDT = F32R

# ms strip table geometry (free-dim element strides inside the SBUF tile)
MS_U = 66
MS_HSTR = MS_U * 32          # 2112 elements per head
MS_TSTR = H * MS_HSTR        # 16896 elements per (main|residual) plane


def build_nc():
    nc = bass.Bass("TRN2", target_bir_lowering=False, debug=False)

    x = nc.dram_tensor("x", [N, D], F32, kind="ExternalInput").ap()
    wq = nc.dram_tensor("wq", [D, H * DK], F32R, kind="ExternalInput").ap()
    wk = nc.dram_tensor("wk", [D, H * DK], F32R, kind="ExternalInput").ap()
    wv = nc.dram_tensor("wv", [D, H * DV], F32R, kind="ExternalInput").ap()
    wo = nc.dram_tensor("wo", [H * DV, DOUT], F32R, kind="ExternalInput").ap()
    ms8d = nc.dram_tensor("ms8", [128, 2, H, MS_U, 32], F8,
                          kind="ExternalInput").ap()
    bo = nc.dram_tensor("bo", [DOUT], F32, kind="ExternalInput").ap()
    gam = nc.dram_tensor("gam", [DOUT], F32, kind="ExternalInput").ap()
    bet = nc.dram_tensor("bet", [DOUT], F32, kind="ExternalInput").ap()
    out = nc.dram_tensor("out", [N, DOUT], F32, kind="ExternalOutput").ap()

    # scratch DRAM for partition-broadcasting the softmax recip rows
    rrd = nc.dram_tensor("rrd", [H, N], F32).ap()

    with tile.TileContext(nc) as tc:
        with (
            tc.tile_pool(name="const", bufs=1) as constp,
            tc.tile_pool(name="big", bufs=1) as bigp,
            tc.tile_pool(name="xin", bufs=3) as xinp,
            tc.tile_pool(name="exps", bufs=3) as expp,
            tc.tile_pool(name="small", bufs=2) as smallp,
            tc.tile_pool(name="yout", bufs=3) as youtp,
            tc.tile_pool(name="ps1", bufs=2, space="PSUM") as ps1p,
            tc.tile_pool(name="ps2", bufs=2, space="PSUM") as ps2p,
        ):
            # ---------------- phase A: x -> xT --------------------------
            # x tiles are DMA'd first: the serial DMA device must deliver
            # them before anything else so PE can start transposing.
            ident = constp.tile([128, 128], F32)
            make_identity(nc, ident)
            xT = bigp.tile([128, 2, N], MDT)
            # x loaded in two batched strided DMAs (one HWDGE slot each):
            # xa[p, nt, d] = x[128*nt + p, d]
            xa = xinp.tile([128, 8, D], F32, tag="xa", bufs=1)
            for half in range(2):
                eng = nc.scalar if half == 0 else nc.sync
                eng.dma_start(
                    out=xa[:, 4 * half:4 * (half + 1), :],
                    in_=bass.AP(tensor=x.tensor, offset=half * 4 * 128 * D,
                                ap=[[D, 128], [128 * D, 4], [1, D]]),
                )

            # batched weight loads: w_sb[p, kt, c] = w[128*kt + p, c]
            wq_sb = constp.tile([128, 2, 256], MDT)
            wk_sb = constp.tile([128, 2, 256], MDT)
            wv_sb = constp.tile([128, 2, 512], MDT)
            wo_sb = constp.tile([128, 4, 256], MDT)
            for dst_sb, wsrc, nk, eng in (
                    (wq_sb, wq, 2, nc.scalar), (wk_sb, wk, 2, nc.sync),
                    (wv_sb, wv, 2, nc.scalar)):
                cols = dst_sb.shape[2]
                eng.dma_start(
                    out=dst_sb,
                    in_=bass.AP(tensor=wsrc.tensor, offset=0,
                                ap=[[cols, 128], [128 * cols, nk], [1, cols]]),
                )

            ms8 = constp.tile([128, 2, H, MS_U, 32], F8)
            nc.gpsimd.dma_start(out=ms8[:, :, 0:2], in_=ms8d[:, :, 0:2])

            for nt in range(8):
                for dt in range(2):
                    pst = ps1p.tile([128, 128], F32, tag="ps1")
                    nc.tensor.transpose(pst, xa[:, nt, 128 * dt:128 * (dt + 1)],
                                        ident)
                    nc.vector.tensor_copy(xT[:, dt, 128 * nt:128 * (nt + 1)], pst)

            id8 = constp.tile([128, 128], F8)
            nc.vector.tensor_copy(id8, ident)

            # ---------------- phase B: QKV proj -------------------------
            qT = bigp.tile([128, 2, N], MDT)
            kT = bigp.tile([128, 2, N], MDT)
            for ic in range(2):
                for dst_sb, w_sb in ((qT, wq_sb), (kT, wk_sb)):
                    for mt in range(2):
                        ps = ps1p.tile([128, 512], F32, tag="ps1")
                        for kt in range(2):
                            nc.tensor.matmul(
                                ps,
                                w_sb[:, kt, 128 * mt:128 * (mt + 1)],
                                xT[:, kt, 512 * ic:512 * (ic + 1)],
                                start=(kt == 0), stop=(kt == 1),
                            )
                        nc.vector.tensor_copy(dst_sb[:, mt, 512 * ic:512 * (ic + 1)], ps)

            # v, augmented with a ones column per head for the softmax
            # denominator: lhsT = [v | 1] -> denominator lands at out row 64.
            va = bigp.tile([128, 8, 8, 65], MDT)
            nc.scalar.activation(va[:, :, :, 64:65],
                                 ident[:, 0:64],
                                 mybir.ActivationFunctionType.Copy,
                                 bias=1.0, scale=0.0)
            for jt in range(8):
                ps = ps1p.tile([128, 512], F32, tag="ps1")
                for kt in range(2):
                    nc.tensor.matmul(
                        ps,
                        xT[:, kt, 128 * jt:128 * (jt + 1)],
                        wv_sb[:, kt, :],
                        start=(kt == 0), stop=(kt == 1),
                    )
                psr = ps.rearrange("p (h v) -> p h v", v=64)
                nc.vector.tensor_copy(va[:, jt, :, 0:64], psr)

            # remaining bias chunks, wo, BN rows — needed progressively
            for hp in (1, 2, 3):
                nc.gpsimd.dma_start(out=ms8[:, :, 2 * hp:2 * hp + 2],
                                    in_=ms8d[:, :, 2 * hp:2 * hp + 2])
            nc.scalar.dma_start(
                out=wo_sb,
                in_=bass.AP(tensor=wo.tensor, offset=0,
                            ap=[[256, 128], [128 * 256, 4], [1, 256]]),
            )

            # BN affine rows (partition-broadcast straight from DRAM):
            # g2 = gamma*c ; b2 = bo*g2 + beta
            g2b = constp.tile([128, DOUT], F32)
            b2b = constp.tile([128, DOUT], F32)
            tmpb = constp.tile([128, DOUT], F32)
            nc.scalar.dma_start(
                out=g2b, in_=bass.AP(tensor=gam.tensor, offset=0,
                                     ap=[[0, 128], [1, DOUT]]))
            nc.scalar.dma_start(
                out=b2b, in_=bass.AP(tensor=bet.tensor, offset=0,
                                     ap=[[0, 128], [1, DOUT]]))
            nc.scalar.dma_start(
                out=tmpb, in_=bass.AP(tensor=bo.tensor, offset=0,
                                      ap=[[0, 128], [1, DOUT]]))
            nc.scalar.mul(g2b, g2b, BN_C)
            nc.vector.tensor_mul(tmpb, tmpb, g2b)
            nc.vector.tensor_add(b2b, b2b, tmpb)

            # ---------------- phase C: attention ------------------------
            # per (h, jt, ic): kq matmul (512c) + DoubleRow bias identity
            # matmul (256c) into the same PSUM region; exp on ACT; attnV
            # one stage behind on PE.
            id8_pair = bass.AP(tensor=id8.tensor, offset=id8.offset,
                               ap=[id8.ap[0], [0, 2], [1, 128]])
            gT = bigp.tile([128, 4, N], MDT)
            for h in range(H):
                mtk = h // 4
                pb = 32 * (h % 4)
                po = ps1p.tile([128, 1024], F32, tag="ps1")
                es_prev = None
                for jt in range(9):
                    if jt < 8:
                        ps = ps2p.tile([128, 1024], F32, tag="st")
                        for ic in range(2):
                            nc.tensor.matmul(
                                ps[:, 512 * ic:512 * (ic + 1)],
                                kT[pb:pb + 32, mtk, 128 * jt:128 * (jt + 1)],
                                qT[pb:pb + 32, mtk, 512 * ic:512 * (ic + 1)],
                                start=True, stop=False,
                                tile_position=(pb, 0),
                            )
                            u0 = 16 * ic + 31 - 4 * jt
                            nc.tensor.matmul(
                                ps[:, 512 * ic:512 * (ic + 1)],
                                id8_pair,
                                bass.AP(tensor=ms8.tensor,
                                        offset=(ms8.offset + h * MS_HSTR
                                                + u0 * 32),
                                        ap=[ms8.ap[0], [MS_TSTR, 2], [1, 512]]),
                                start=False, stop=True,
                                perf_mode=mybir.MatmulPerfMode.DoubleRow,
                            )
                        es = expp.tile([128, 1024], MDT, tag="es")
                        nc.scalar.activation(es, ps,
                                             mybir.ActivationFunctionType.Exp,
                                             scale=SCALE)
                    # attnV one stage behind so PE never stalls on exp
                    if jt > 0:
                        for ic in range(2):
                            nc.tensor.matmul(
                                po[0:65, 512 * ic:512 * (ic + 1)],
                                va[:, jt - 1, h, :],
                                es_prev[:, 512 * ic:512 * (ic + 1)],
                                start=(jt == 1), stop=(jt == 8),
                            )
                    es_prev = es
                # normalize: out/denominator (DMA-broadcast the recip row
                # across partitions via DRAM; SBUF APs need nonzero
                # partition step)
                rr = smallp.tile([1, N], F32, tag="rr", bufs=1)
                nc.vector.reciprocal(rr, po[64:65, :])
                nc.sync.dma_start(out=rrd[h, :].unsqueeze(0), in_=rr)
                rrb = smallp.tile([64, N], F32, tag="rrb", bufs=1)
                nc.sync.dma_start(
                    out=rrb,
                    in_=bass.AP(tensor=rrd.tensor, offset=h * N,
                                ap=[[0, 64], [1, N]]),
                )
                nc.vector.tensor_mul(
                    gT[64 * (h % 2):64 * (h % 2) + 64, h // 2, :],
                    po[0:64, :],
                    rrb,
                )

            # ------- phase D/E: gelu + out proj + BN, pipelined ---------
            # gelu is applied per 128-column block (all 4 kt chunks of that
            # block in one ACT op) so each out-proj tile can start right
            # after its own gelu, overlapping ACT and PE in the tail.
            yt = youtp.tile([128, 8, DOUT], F32, tag="yt", bufs=1)
            for it in range(8):
                gsl = bass.AP(tensor=gT.tensor,
                              offset=gT.offset + 128 * it,
                              ap=[gT.ap[0], [N, 4], [1, 128]])
                nc.scalar.activation(gsl, gsl,
                                     mybir.ActivationFunctionType.Gelu)
                ps = ps1p.tile([128, 512], F32, tag="ps1")
                for kt in range(4):
                    nc.tensor.matmul(
                        ps[:, 0:256],
                        gT[:, kt, 128 * it:128 * (it + 1)],
                        wo_sb[:, kt, :],
                        start=(kt == 0), stop=(kt == 3),
                    )
                nc.vector.tensor_mul(yt[:, it, :], ps[:, 0:256], g2b)
                nc.vector.tensor_add(yt[:, it, :], yt[:, it, :], b2b)
                if it % 4 == 3:
                    # batched store: out[128*nt + p, :] = yt[p, nt, :]
                    eng = nc.sync if it == 3 else nc.scalar
                    eng.dma_start(
                        out=bass.AP(tensor=out.tensor,
                                    offset=(it - 3) * 128 * DOUT,
                                    ap=[[DOUT, 128], [128 * DOUT, 4],
                                        [1, DOUT]]),
                        in_=yt[:, it - 3:it + 1, :],
                    )

    _split_excess_waits(nc)
    return nc


def _split_excess_waits(nc):
    """walrus rejects >1 sem-wait per instruction ("Too many sync wait
    commands"); unroll extras into a chain of single-wait same-engine
    NoOps directly before the instruction."""
    ctr = 0
    for fn in nc.m.functions:
        for blk in fn.blocks:
            out = []
            for inst in blk.instructions:
                si = inst.sync_info
                if si is not None and len(si.on_wait) > 1:
                    for w in si.on_wait[:-1]:
                        nop = mybir.InstNoOp(name=f"waitnop-{ctr}")
                        ctr += 1
                        nop.engine = inst.engine
                        nop.sync_info = mybir.SyncInfo(
                            on_wait=[w], on_update=[])
                        out.append(nop)
                    inst.sync_info = mybir.SyncInfo(
                        on_wait=[si.on_wait[-1]], on_update=list(si.on_update))
                out.append(inst)
            blk.instructions = out


def _build_ms8(pos_emb: np.ndarray) -> np.ndarray:
    """Host-precompute the fp8 (main, residual) bias strip table.

    table[(g,cj), t, h, u, ci] approximates 32*E_h[|u-31-g|, |ci-cj|]
    (main + residual), where E = pos_emb.reshape(32, 32, H).
    """
    import ml_dtypes

    E = np.asarray(pos_emb, dtype=np.float32).reshape(32, 32, H)
    T = E.transpose(2, 0, 1)                                   # [h, a, b]
    g = np.arange(4)
    u = np.arange(MS_U)
    a_idx = np.abs(u[None, :] - 31 - g[:, None]).clip(0, 31)   # [4, 66]
    c = np.arange(32)
    b_idx = np.abs(c[None, :] - c[:, None])                    # [cj, ci]
    tmp = T[:, a_idx]                                          # [h, 4, 66, b]
    tab = tmp[:, :, :, b_idx]                                  # [h, 4, 66, cj, ci]
    # -> [(g, cj), h, u, ci]
    arr = np.ascontiguousarray(tab.transpose(1, 3, 0, 2, 4)).reshape(
        4 * 32, H, MS_U, 32) * np.float32(DK)
    main = arr.astype(ml_dtypes.float8_e4m3)
    res = (arr - main.astype(np.float32)).astype(ml_dtypes.float8_e4m3)
    return np.ascontiguousarray(
        np.stack([main, res], axis=1))                         # [128, 2, h, u, ci]


_NC_CACHE = None


def kernel(**inputs) -> np.ndarray:
    global _NC_CACHE
    x = np.ascontiguousarray(inputs["x"], dtype=np.float32)        # (8,32,32,256)
    shared = {
        "wq": np.ascontiguousarray(inputs["Wq"], dtype=np.float32),
        "wk": np.ascontiguousarray(inputs["Wk"], dtype=np.float32),
        "wv": np.ascontiguousarray(inputs["Wv"], dtype=np.float32),
        "wo": np.ascontiguousarray(inputs["Wo"], dtype=np.float32),
        "ms8": _build_ms8(inputs["pos_emb"]),
        "bo": np.ascontiguousarray(inputs["bo"], dtype=np.float32),
        "gam": np.ascontiguousarray(inputs["gamma"], dtype=np.float32),
        "bet": np.ascontiguousarray(inputs["beta"], dtype=np.float32),
    }
    in_maps = []
    for c in range(NCORES):
        m = dict(shared)
        m["x"] = np.ascontiguousarray(x[c].reshape(N, D))
        in_maps.append(m)

    if _NC_CACHE is None:
        _NC_CACHE = build_nc()
    res = run_bass_kernel_spmd(_NC_CACHE, in_maps, core_ids=list(range(NCORES)))
    outs = [res.results[c]["out"].reshape(FM, FM, DOUT) for c in range(NCORES)]
    return np.stack(outs, axis=0)


if __name__ == "__main__":
    build_nc()
    print("build ok")
